# revision 8
# baseline (speedup 1.0000x reference)
"""CWS (Chinese word segmentation) greedy-agenda model kernel for trn2.

Strategy (per sharding hint): data-parallel across sentences. The expensive,
fully-parallel part of the model — the per-word-length reset gate and the
composition projection for every (position, word-length) pair — runs as a
Bass/Tile SPMD kernel on 8 NeuronCores (batch B=128 sharded 16/core, small
parameters replicated).  Key identity: the reset/composition math for step t,
window-age c depends only on chars[b, t-c], so proj[b, t', w, :] is computed
once per position t' instead of once per (t, c) pair — a 4x FLOP reduction
vs the reference einsums.

Device kernel works entirely in transposed [feature, position] layout:
  gT[d, p]   = sigmoid(reset_W[w].T @ embT + reset_b[w])   (bias on partitions,
                                                            fused into ACT)
  gT        *= embT                                         (VectorE)
  projT[e,p] = tanh(com_W.T @ (gT) + com_b)                 (bias fused)
No on-device transposes are needed; matmul contraction is along partitions.

The remaining recurrence (score -> argmax -> LSTM -> buffer shift) is a tiny,
strictly-sequential chain over T=256 steps, vectorized over B on host using
the precomputed word tensors.  If the device path fails for any reason the
kernel falls back to a numerically-identical host computation.
"""

import numpy as np

B, T, L, DC, DW, H, V = 128, 256, 4, 128, 128, 256, 6000
NEG = -1e30
N_CORES = 8
BL = B // N_CORES          # 16 sentences per core
P = T * BL                 # 4096 positions per core
CHUNK = 512                # positions per matmul (max fp32 moving dim)
NCHUNK = P // CHUNK


def _sigmoid(x):
    out = np.empty_like(x)
    np.negative(x, out=out)
    np.exp(out, out=out)
    out += 1.0
    np.reciprocal(out, out=out)
    return out


def _proj_host(chars, char_emb, reset_W, reset_b, com_W, com_b):
    emb = char_emb[chars]                       # [B, T, DC]
    flat = emb.reshape(B * T, DC)
    proj = np.empty((L, B * T, DW), np.float32)
    for w in range(L):
        g = _sigmoid(flat @ reset_W[w] + reset_b[w])
        g *= flat
        proj[w] = np.tanh(g @ com_W + com_b)
    return proj.reshape(L, B, T, DW)


def _build_bass(trace=False):
    """Raw Bass SPMD program (explicit semaphores; one condition per wait —
    this walrus build rejects instructions carrying multiple attached waits,
    so TileContext is not usable here)."""
    import contextlib

    import concourse.bass as bass
    from concourse import mybir

    nc = bass.Bass()
    f32 = mybir.dt.float32
    AF = mybir.ActivationFunctionType
    NPAR = L * DC + DW + L + 1  # 645 packed param columns
    embT_in = nc.dram_tensor("embT", [DC, P], f32, kind="ExternalInput")
    par_in = nc.dram_tensor("params", [DC, NPAR], f32, kind="ExternalInput")
    proj_out = nc.dram_tensor("projT", [L, DW, P], f32, kind="ExternalOutput")
    cw_off = L * DC
    rb_off = cw_off + DW
    cb_off = rb_off + L
    K = NCHUNK * L  # 32 pipeline iterations, k = i*L + w

    ctx = contextlib.ExitStack()
    with ctx:
        par = ctx.enter_context(nc.sbuf_tensor([DC, NPAR], f32))
        embT = ctx.enter_context(nc.sbuf_tensor([DC, NCHUNK, CHUNK], f32))
        g = ctx.enter_context(nc.sbuf_tensor([DC, 2, CHUNK], f32))
        pj = ctx.enter_context(nc.sbuf_tensor([DW, 4, CHUNK], f32))
        gp = ctx.enter_context(nc.psum_tensor([DC, 2, CHUNK], f32))
        pp = ctx.enter_context(nc.psum_tensor([DW, 2, CHUNK], f32))
        dma_in = ctx.enter_context(nc.semaphore())
        dma_out = ctx.enter_context(nc.semaphore())
        pe1 = ctx.enter_context(nc.semaphore())
        pe2 = ctx.enter_context(nc.semaphore())
        act1 = ctx.enter_context(nc.semaphore())
        act2 = ctx.enter_context(nc.semaphore())
        dve = ctx.enter_context(nc.semaphore())
        blk = ctx.enter_context(nc.Block())

        @blk.sync
        def _(sync):
            sync.dma_start(out=par[:, :], in_=par_in[:, :]).then_inc(dma_in, 16)
            for i in range(NCHUNK):
                sync.dma_start(
                    out=embT[:, i, :], in_=embT_in[:, bass.ts(i, CHUNK)]
                ).then_inc(dma_in, 16)
            for k in range(K):
                i, w = divmod(k, L)
                sync.wait_ge(act2, k + 1)
                sync.dma_start(
                    out=proj_out[w, :, bass.ts(i, CHUNK)], in_=pj[:, k % 4, :]
                ).then_inc(dma_out, 16)

        @blk.tensor
        def _(tensor):
            for k in range(K):
                i, w = divmod(k, L)
                if w == 0:
                    tensor.wait_ge(dma_in, 16 * (i + 2))
                nc.tensor.matmul(
                    gp[:, k % 2, :], par[:, bass.ts(w, DC)], embT[:, i, :],
                    start=True, stop=True,
                ).then_inc(pe1, 1)
                if k >= 2:
                    tensor.wait_ge(act2, k - 1)
                tensor.wait_ge(dve, k + 1)
                nc.tensor.matmul(
                    pp[:, k % 2, :], par[:, cw_off : cw_off + DW], g[:, k % 2, :],
                    start=True, stop=True,
                ).then_inc(pe2, 1)

        @blk.scalar
        def _(scalar):
            for k in range(K):
                i, w = divmod(k, L)
                scalar.wait_ge(pe1, k + 1)
                nc.scalar.activation(
                    g[:, k % 2, :], gp[:, k % 2, :], AF.Sigmoid,
                    bias=par[:, rb_off + w : rb_off + w + 1],
                ).then_inc(act1, 1)
                scalar.wait_ge(pe2, k + 1)
                if k >= 4:
                    scalar.wait_ge(dma_out, 16 * (k - 3))
                nc.scalar.activation(
                    pj[:, k % 4, :], pp[:, k % 2, :], AF.Tanh,
                    bias=par[:, cb_off : cb_off + 1],
                ).then_inc(act2, 1)

        @blk.vector
        def _(vector):
            for k in range(K):
                i, w = divmod(k, L)
                vector.wait_ge(act1, k + 1)
                nc.vector.tensor_mul(
                    g[:, k % 2, :], g[:, k % 2, :], embT[:, i, :]
                ).then_inc(dve, 1)
    return nc


def _try_device_proj(chars, char_emb, reset_W, reset_b, com_W, com_b,
                     trace=False):
    try:
        from concourse.bass_utils import run_bass_kernel_spmd

        nc = _build_bass()
        emb_full = char_emb[chars]              # [B, T, DC]
        params = np.ascontiguousarray(
            np.concatenate(
                [
                    reset_W.transpose(1, 0, 2).reshape(DC, L * DC),  # [d, w*dhat]
                    com_W,                                           # [d, e]
                    reset_b.T,                                       # [dhat, w]
                    com_b[:, None],                                  # [e, 1]
                ],
                axis=1,
            ),
            np.float32,
        )
        in_maps = []
        for c in range(N_CORES):
            shard = emb_full[c * BL : (c + 1) * BL].reshape(P, DC)
            in_maps.append({
                "embT": np.ascontiguousarray(shard.T, np.float32),
                "params": params,
            })
        res = run_bass_kernel_spmd(nc, in_maps, core_ids=list(range(N_CORES)),
                                   trace=trace)
        proj = np.empty((L, B, T, DW), np.float32)
        for c in range(N_CORES):
            pr = res.results[c]["projT"]        # [L, DW, P]
            proj[:, c * BL : (c + 1) * BL] = (
                pr.reshape(L, DW, BL, T).transpose(0, 2, 3, 1))
        if trace:
            print(f"HW exec time: {res.exec_time_ns} ns")
        return proj
    except Exception as e:  # pragma: no cover
        import traceback
        traceback.print_exc()
        print(f"[kernel] device path failed ({type(e).__name__}); host fallback")
        return None


def _word_from_proj(proj):
    """word[b, t, w, :] = mean_{c<=w} proj[w, b, t-c, :]."""
    word = np.zeros((B, T, L, DW), np.float32)
    for w in range(L):
        acc = proj[w].copy()
        for c in range(1, w + 1):
            acc[:, c:] += proj[w][:, :-c]
        word[:, :, w, :] = acc / np.float32(w + 1)
    return word


def kernel(chars, char_emb, reset_W, reset_b, com_W, com_b, lstm_kernel,
           lstm_bias, pred_W, pred_b, score_U, bos):
    chars = np.asarray(chars)
    char_emb = np.asarray(char_emb, np.float32)
    reset_W = np.asarray(reset_W, np.float32)
    reset_b = np.asarray(reset_b, np.float32)
    com_W = np.asarray(com_W, np.float32)
    com_b = np.asarray(com_b, np.float32)
    lstm_kernel = np.asarray(lstm_kernel, np.float32)
    lstm_bias = np.asarray(lstm_bias, np.float32)
    pred_W = np.asarray(pred_W, np.float32)
    pred_b = np.asarray(pred_b, np.float32)
    score_U = np.asarray(score_U, np.float32)
    bos = np.asarray(bos, np.float32)

    proj = _try_device_proj(chars, char_emb, reset_W, reset_b, com_W, com_b)
    if proj is None:
        proj = _proj_host(chars, char_emb, reset_W, reset_b, com_W, com_b)
    word = _word_from_proj(proj)                # [B, T, L, DW]

    # ---- sequential agenda recurrence (host, vectorized over B) ----
    Kx = lstm_kernel[:DW]
    Kh = lstm_kernel[DW:]

    def lstm(x, c, h):
        z = x @ Kx + h @ Kh + lstm_bias
        i = z[:, :H]; j = z[:, H:2*H]; f = z[:, 2*H:3*H]; o = z[:, 3*H:]
        ncell = c * _sigmoid(f) + _sigmoid(i) * np.tanh(j)
        nh = np.tanh(ncell) * _sigmoid(o)
        return ncell, nh

    c0 = np.zeros((B, H), np.float32)
    h0 = np.zeros((B, H), np.float32)
    x0 = np.broadcast_to(bos, (B, DW))
    c1, h1 = lstm(x0, c0, h0)
    pred0 = np.tanh(h1 @ pred_W + pred_b)
    buf_pred = np.repeat(pred0[:, None, :], L, axis=1)
    buf_c = np.repeat(c1[:, None, :], L, axis=1)
    buf_h = np.repeat(h1[:, None, :], L, axis=1)

    wlens = np.arange(1, L + 1)
    bidx = np.arange(B)
    scores_out = np.empty((T, B), np.float32)
    wl_out = np.empty((T, B), np.int32)
    for t in range(T):
        wt = word[:, t]                          # [B, L, DW]
        score = np.einsum("ble,ble->bl", buf_pred + score_U, wt).astype(np.float32)
        score = np.where((wlens <= t + 1)[None, :], score, np.float32(NEG))
        best = np.argmax(score, axis=1)
        word_b = wt[bidx, best]
        c_prev = buf_c[bidx, best]
        h_prev = buf_h[bidx, best]
        ncell, nh = lstm(word_b, c_prev, h_prev)
        npred = np.tanh(nh @ pred_W + pred_b)
        buf_pred = np.concatenate([npred[:, None], buf_pred[:, :-1]], axis=1)
        buf_c = np.concatenate([ncell[:, None], buf_c[:, :-1]], axis=1)
        buf_h = np.concatenate([nh[:, None], buf_h[:, :-1]], axis=1)
        scores_out[t] = score[bidx, best]
        wl_out[t] = best + 1

    return scores_out.T.copy(), wl_out.T.copy()


if __name__ == "__main__":
    d = dict(np.load("/tmp/inputs.npz"))
    s, w = kernel(**d)
    print(s.shape, w.shape)


# revision 9
# speedup vs baseline: 1.4027x; 1.4027x over previous
"""CWS (Chinese word segmentation) greedy-agenda model kernel for trn2.

Strategy (per sharding hint): data-parallel across sentences. The expensive,
fully-parallel part of the model — the per-word-length reset gate and the
composition projection for every (position, word-length) pair — runs as a
Bass/Tile SPMD kernel on 8 NeuronCores (batch B=128 sharded 16/core, small
parameters replicated).  Key identity: the reset/composition math for step t,
window-age c depends only on chars[b, t-c], so proj[b, t', w, :] is computed
once per position t' instead of once per (t, c) pair — a 4x FLOP reduction
vs the reference einsums.

Device kernel works entirely in transposed [feature, position] layout:
  gT[d, p]   = sigmoid(reset_W[w].T @ embT + reset_b[w])   (bias on partitions,
                                                            fused into ACT)
  gT        *= embT                                         (VectorE)
  projT[e,p] = tanh(com_W.T @ (gT) + com_b)                 (bias fused)
No on-device transposes are needed; matmul contraction is along partitions.

The remaining recurrence (score -> argmax -> LSTM -> buffer shift) is a tiny,
strictly-sequential chain over T=256 steps, vectorized over B on host using
the precomputed word tensors.  If the device path fails for any reason the
kernel falls back to a numerically-identical host computation.
"""

import numpy as np

B, T, L, DC, DW, H, V = 128, 256, 4, 128, 128, 256, 6000
NEG = -1e30
N_CORES = 8
BL = B // N_CORES          # 16 sentences per core
P = T * BL                 # 4096 positions per core
CHUNK = 512                # positions per matmul (max fp32 moving dim)
NCHUNK = P // CHUNK


def _sigmoid(x):
    out = np.empty_like(x)
    np.negative(x, out=out)
    np.exp(out, out=out)
    out += 1.0
    np.reciprocal(out, out=out)
    return out


def _proj_host(chars, char_emb, reset_W, reset_b, com_W, com_b):
    emb = char_emb[chars]                       # [B, T, DC]
    flat = emb.reshape(B * T, DC)
    proj = np.empty((L, B * T, DW), np.float32)
    for w in range(L):
        g = _sigmoid(flat @ reset_W[w] + reset_b[w])
        g *= flat
        proj[w] = np.tanh(g @ com_W + com_b)
    return proj.reshape(L, B, T, DW)


def _build_bass(trace=False):
    """Raw Bass SPMD program (explicit semaphores; one condition per wait —
    this walrus build rejects instructions carrying multiple attached waits,
    so TileContext is not usable here)."""
    import contextlib

    import concourse.bass as bass
    from concourse import mybir

    nc = bass.Bass()
    f32 = mybir.dt.float32
    AF = mybir.ActivationFunctionType
    NPAR = L * DC + DW + L + 1  # 645 packed param columns
    embT_in = nc.dram_tensor("embT", [DC, P], f32, kind="ExternalInput")
    par_in = nc.dram_tensor("params", [DC, NPAR], f32, kind="ExternalInput")
    proj_out = nc.dram_tensor("projT", [L, DW, P], f32, kind="ExternalOutput")
    cw_off = L * DC
    rb_off = cw_off + DW
    cb_off = rb_off + L
    K = NCHUNK * L  # 32 pipeline iterations, k = i*L + w

    ctx = contextlib.ExitStack()
    with ctx:
        par = ctx.enter_context(nc.sbuf_tensor([DC, NPAR], f32))
        embT = ctx.enter_context(nc.sbuf_tensor([DC, NCHUNK, CHUNK], f32))
        g = ctx.enter_context(nc.sbuf_tensor([DC, 4, CHUNK], f32))
        pj = ctx.enter_context(nc.sbuf_tensor([DW, 4, CHUNK], f32))
        gp = ctx.enter_context(nc.psum_tensor([DC, 4, CHUNK], f32))
        pp = ctx.enter_context(nc.psum_tensor([DW, 2, CHUNK], f32))
        dma_in = ctx.enter_context(nc.semaphore())
        dma_out = ctx.enter_context(nc.semaphore())
        pe1 = ctx.enter_context(nc.semaphore())
        pe2 = ctx.enter_context(nc.semaphore())
        act1 = ctx.enter_context(nc.semaphore())
        act2 = ctx.enter_context(nc.semaphore())
        dve = ctx.enter_context(nc.semaphore())
        blk = ctx.enter_context(nc.Block())

        # 5-stage pipeline over k = i*L + w:
        #   MM1(k) -> sig(k) -> mul(k) -> MM2(k) -> tanh(k) -> outdma(k)
        # PE runs MM1 one iteration ahead of MM2; ACT issues sig(k+1) before
        # tanh(k); each wait is a standalone single-condition instruction.
        @blk.sync
        def _(sync):
            sync.dma_start(out=par[:, :], in_=par_in[:, :]).then_inc(dma_in, 16)
            for i in range(NCHUNK):
                sync.dma_start(
                    out=embT[:, i, :], in_=embT_in[:, bass.ts(i, CHUNK)]
                ).then_inc(dma_in, 16)
            for k in range(K):
                i, w = divmod(k, L)
                sync.wait_ge(act2, k + 1)
                sync.dma_start(
                    out=proj_out[w, :, bass.ts(i, CHUNK)], in_=pj[:, k % 4, :]
                ).then_inc(dma_out, 16)

        def emit_mm2(tensor, k):
            if k >= 2:
                tensor.wait_ge(act2, k - 1)      # pp[k%2] free (tanh(k-2))
            tensor.wait_ge(dve, k + 1)           # mul(k) done
            nc.tensor.matmul(
                pp[:, k % 2, :], par[:, cw_off : cw_off + DW], g[:, k % 4, :],
                start=True, stop=True,
            ).then_inc(pe2, 1)

        @blk.tensor
        def _(tensor):
            for k in range(K):
                i, w = divmod(k, L)
                if w == 0:
                    tensor.wait_ge(dma_in, 16 * (i + 2))
                if k >= 4:
                    tensor.wait_ge(act1, k - 3)  # gp[k%4] free (sig(k-4))
                nc.tensor.matmul(
                    gp[:, k % 4, :], par[:, bass.ts(w, DC)], embT[:, i, :],
                    start=True, stop=True,
                ).then_inc(pe1, 1)
                if k >= 1:
                    emit_mm2(tensor, k - 1)
            emit_mm2(tensor, K - 1)

        def emit_sig(scalar, k):
            w = k % L
            if k >= 4:
                scalar.wait_ge(dve, k - 2)       # g[k%4] free (mul(k-4))
            scalar.wait_ge(pe1, k + 1)           # MM1(k) done
            nc.scalar.activation(
                g[:, k % 4, :], gp[:, k % 4, :], AF.Sigmoid,
                bias=par[:, rb_off + w : rb_off + w + 1],
            ).then_inc(act1, 1)

        @blk.scalar
        def _(scalar):
            emit_sig(scalar, 0)
            for k in range(K):
                if k + 1 < K:
                    emit_sig(scalar, k + 1)
                scalar.wait_ge(pe2, k + 1)       # MM2(k) done
                if k >= 4:
                    scalar.wait_ge(dma_out, 16 * (k - 3))  # pj[k%4] free
                nc.scalar.activation(
                    pj[:, k % 4, :], pp[:, k % 2, :], AF.Tanh,
                    bias=par[:, cb_off : cb_off + 1],
                ).then_inc(act2, 1)

        @blk.vector
        def _(vector):
            for k in range(K):
                i, w = divmod(k, L)
                vector.wait_ge(act1, k + 1)
                nc.vector.tensor_mul(
                    g[:, k % 4, :], g[:, k % 4, :], embT[:, i, :]
                ).then_inc(dve, 1)
    return nc


def _try_device_proj(chars, char_emb, reset_W, reset_b, com_W, com_b,
                     trace=False):
    try:
        from concourse.bass_utils import run_bass_kernel_spmd

        nc = _build_bass()
        emb_full = char_emb[chars]              # [B, T, DC]
        params = np.ascontiguousarray(
            np.concatenate(
                [
                    reset_W.transpose(1, 0, 2).reshape(DC, L * DC),  # [d, w*dhat]
                    com_W,                                           # [d, e]
                    reset_b.T,                                       # [dhat, w]
                    com_b[:, None],                                  # [e, 1]
                ],
                axis=1,
            ),
            np.float32,
        )
        in_maps = []
        for c in range(N_CORES):
            shard = emb_full[c * BL : (c + 1) * BL].reshape(P, DC)
            in_maps.append({
                "embT": np.ascontiguousarray(shard.T, np.float32),
                "params": params,
            })
        res = run_bass_kernel_spmd(nc, in_maps, core_ids=list(range(N_CORES)),
                                   trace=trace)
        proj = np.empty((L, B, T, DW), np.float32)
        for c in range(N_CORES):
            pr = res.results[c]["projT"]        # [L, DW, P]
            proj[:, c * BL : (c + 1) * BL] = (
                pr.reshape(L, DW, BL, T).transpose(0, 2, 3, 1))
        if trace:
            print(f"HW exec time: {res.exec_time_ns} ns")
        return proj
    except Exception as e:  # pragma: no cover
        import traceback
        traceback.print_exc()
        print(f"[kernel] device path failed ({type(e).__name__}); host fallback")
        return None


def _word_from_proj(proj):
    """word[b, t, w, :] = mean_{c<=w} proj[w, b, t-c, :]."""
    word = np.zeros((B, T, L, DW), np.float32)
    for w in range(L):
        acc = proj[w].copy()
        for c in range(1, w + 1):
            acc[:, c:] += proj[w][:, :-c]
        word[:, :, w, :] = acc / np.float32(w + 1)
    return word


def kernel(chars, char_emb, reset_W, reset_b, com_W, com_b, lstm_kernel,
           lstm_bias, pred_W, pred_b, score_U, bos):
    chars = np.asarray(chars)
    char_emb = np.asarray(char_emb, np.float32)
    reset_W = np.asarray(reset_W, np.float32)
    reset_b = np.asarray(reset_b, np.float32)
    com_W = np.asarray(com_W, np.float32)
    com_b = np.asarray(com_b, np.float32)
    lstm_kernel = np.asarray(lstm_kernel, np.float32)
    lstm_bias = np.asarray(lstm_bias, np.float32)
    pred_W = np.asarray(pred_W, np.float32)
    pred_b = np.asarray(pred_b, np.float32)
    score_U = np.asarray(score_U, np.float32)
    bos = np.asarray(bos, np.float32)

    proj = _try_device_proj(chars, char_emb, reset_W, reset_b, com_W, com_b)
    if proj is None:
        proj = _proj_host(chars, char_emb, reset_W, reset_b, com_W, com_b)
    word = _word_from_proj(proj)                # [B, T, L, DW]

    # ---- sequential agenda recurrence (host, vectorized over B) ----
    Kx = lstm_kernel[:DW]
    Kh = lstm_kernel[DW:]

    def lstm(x, c, h):
        z = x @ Kx + h @ Kh + lstm_bias
        i = z[:, :H]; j = z[:, H:2*H]; f = z[:, 2*H:3*H]; o = z[:, 3*H:]
        ncell = c * _sigmoid(f) + _sigmoid(i) * np.tanh(j)
        nh = np.tanh(ncell) * _sigmoid(o)
        return ncell, nh

    c0 = np.zeros((B, H), np.float32)
    h0 = np.zeros((B, H), np.float32)
    x0 = np.broadcast_to(bos, (B, DW))
    c1, h1 = lstm(x0, c0, h0)
    pred0 = np.tanh(h1 @ pred_W + pred_b)
    buf_pred = np.repeat(pred0[:, None, :], L, axis=1)
    buf_c = np.repeat(c1[:, None, :], L, axis=1)
    buf_h = np.repeat(h1[:, None, :], L, axis=1)

    wlens = np.arange(1, L + 1)
    bidx = np.arange(B)
    scores_out = np.empty((T, B), np.float32)
    wl_out = np.empty((T, B), np.int32)
    for t in range(T):
        wt = word[:, t]                          # [B, L, DW]
        score = np.einsum("ble,ble->bl", buf_pred + score_U, wt).astype(np.float32)
        score = np.where((wlens <= t + 1)[None, :], score, np.float32(NEG))
        best = np.argmax(score, axis=1)
        word_b = wt[bidx, best]
        c_prev = buf_c[bidx, best]
        h_prev = buf_h[bidx, best]
        ncell, nh = lstm(word_b, c_prev, h_prev)
        npred = np.tanh(nh @ pred_W + pred_b)
        buf_pred = np.concatenate([npred[:, None], buf_pred[:, :-1]], axis=1)
        buf_c = np.concatenate([ncell[:, None], buf_c[:, :-1]], axis=1)
        buf_h = np.concatenate([nh[:, None], buf_h[:, :-1]], axis=1)
        scores_out[t] = score[bidx, best]
        wl_out[t] = best + 1

    return scores_out.T.copy(), wl_out.T.copy()


if __name__ == "__main__":
    d = dict(np.load("/tmp/inputs.npz"))
    s, w = kernel(**d)
    print(s.shape, w.shape)
